# revision 1
# baseline (speedup 1.0000x reference)
"""Fused ConvBNReLU1D (kernel_size=1) + per-tensor po2 weight/bias fake-quant
+ QuantReLU(8-bit unsigned) output fake-quant, on 8 Trainium2 NeuronCores.

Strategy
--------
- Host: quantize W/b (per-tensor po2 scales, depends only on W/b - "precomputed
  scale" option from the sharding hint).
- Device (SPMD, data-parallel over batch B=32 -> 4 batches/core):
  Phase A: pointwise GEMM y = relu(Wq @ x + bq) with float32r matmuls
           (fp32 operands truncated to FP22 in the PE; 1 cycle/row for
           free-dim >= 256, i.e. full bf16 speed with 13 mantissa bits).
           y stays resident in SBUF (128 KiB/partition); per-chunk running
           maxes tracked on the vector engine.
  - AllReduce(max) of the per-partition max vector across the 8 cores
    (the output scale s = max(y)/255 is global).
  Phase B: out = round(y/s)*s elementwise, with round-to-nearest-even done
           via the +/- 1.5*2^23 magic-constant trick (matches jnp.round),
           then DMA out.
"""

import os
import sys
from contextlib import ExitStack

import numpy as np

for _p in ("/opt/trn_rl_repo", os.path.expanduser("~/.axon_site/_ro/trn_rl_repo")):
    if os.path.isdir(_p) and _p not in sys.path:
        sys.path.insert(0, _p)

import concourse.bacc as bacc
import concourse.mybir as mybir
import concourse.tile as tile
from concourse.bass_utils import run_bass_kernel_spmd

P = 128
B, CIN, COUT, N = 32, 512, 512, 2048
NCORES = 8
BSH = B // NCORES          # batches per core
NT = 512                   # matmul free dim (= one PSUM bank of fp32)
KT = CIN // P              # 4 contraction tiles
MT = COUT // P             # 4 output-row tiles
NJ = N // NT               # 4 n-windows per batch
NCH = BSH * NJ             # 16 (batch, n-window) chunks per core
CH2 = MT * NT              # columns of y per chunk (2048)
MAGIC = 12582912.0         # 1.5 * 2^23: RNE rounding for t in [0, 2^22)
QMAX_S = 127.0
QMAX_U = 255.0

_cache = {}
LAST_RESULT = None         # BassKernelResults of the most recent run (test.py)


def _build():
    f32 = mybir.dt.float32
    f32r = mybir.dt.float32r
    Relu = mybir.ActivationFunctionType.Relu
    Copy = mybir.ActivationFunctionType.Copy
    X = mybir.AxisListType.X
    Alu = mybir.AluOpType

    nc = bacc.Bacc(
        "TRN2",
        target_bir_lowering=False,
        debug=False,
        enable_asserts=False,
        num_devices=NCORES,
    )
    xs = nc.dram_tensor("xs", [BSH, CIN, N], f32r, kind="ExternalInput")
    wT = nc.dram_tensor("wT", [CIN, COUT], f32r, kind="ExternalInput")
    bqv = nc.dram_tensor("bqv", [P, MT], f32, kind="ExternalInput")
    out = nc.dram_tensor("out", [BSH, COUT, N], f32, kind="ExternalOutput")

    with tile.TileContext(nc) as tc, ExitStack() as ctx:
        const = ctx.enter_context(tc.tile_pool(name="const", bufs=1))
        xpool = ctx.enter_context(tc.tile_pool(name="xp", bufs=3))
        ypool = ctx.enter_context(tc.tile_pool(name="yp", bufs=1))
        pspool = ctx.enter_context(tc.tile_pool(name="ps", bufs=7, space="PSUM"))
        psb = ctx.enter_context(tc.tile_pool(name="psb", bufs=1, space="PSUM"))
        tpool = ctx.enter_context(tc.tile_pool(name="tp", bufs=3))
        dram = ctx.enter_context(tc.tile_pool(name="dram", bufs=1, space="DRAM"))

        def load_x_chunk(c):
            bb, j = divmod(c, NJ)
            xt = xpool.tile([P, KT * NT], f32r)
            # per-k-slice DMAs: matmul k waits only on its own 256 KiB slice,
            # so the PE never stalls (and never drops out of warm p-state)
            # at a chunk boundary
            for k in range(KT):
                nc.sync.dma_start(
                    out=xt[:, k * NT:(k + 1) * NT],
                    in_=xs[bb, k * P:(k + 1) * P, j * NT:(j + 1) * NT],
                )
            return xt

        # prefetch the first x chunk before the (larger) weight load so the
        # first matmul's inputs land as early as possible
        xtiles = {0: load_x_chunk(0)}

        # Weights: lhsT tile (k, m) = Wq.T[k*128:(k+1)*128, m*128:(m+1)*128],
        # packed at column (k*MT+m)*P, all in one 1 MiB DMA
        wq = const.tile([P, KT * MT * P], f32r)
        nc.sync.dma_start(
            out=wq[:, :].rearrange("p (k m q) -> p k m q", k=KT, m=MT),
            in_=wT[:, :].rearrange("(k p) (m q) -> p k m q", p=P, q=P),
        )
        bias = const.tile([P, MT], f32)
        nc.sync.dma_start(out=bias[:], in_=bqv[:, :])

        ybig = ypool.tile([P, NCH * CH2], f32)
        maxb = const.tile([P, NCH * MT], f32)

        # ---- Phase A: y = relu(Wq @ x + bq), track per-column-block maxes
        for c in range(NCH):
            xt = xtiles.pop(c) if c in xtiles else load_x_chunk(c)
            for m in range(MT):
                ps = pspool.tile([P, NT], f32)
                for k in range(KT):
                    nc.tensor.matmul(
                        ps[:],
                        wq[:, (k * MT + m) * P:(k * MT + m + 1) * P],
                        xt[:, k * NT:(k + 1) * NT],
                        start=(k == 0),
                        stop=(k == KT - 1),
                    )
                col = (c * MT + m) * NT
                nc.scalar.activation(
                    ybig[:, col:col + NT], ps[:], Relu, bias=bias[:, m:m + 1]
                )
                nc.vector.reduce_max(
                    maxb[:, c * MT + m:c * MT + m + 1],
                    ybig[:, col:col + NT],
                    axis=X,
                )

        # ---- Global max across cores (scale is global)
        mloc = const.tile([P, 1], f32)
        nc.vector.reduce_max(mloc[:], maxb[:], axis=X)
        cc_in = dram.tile([1, P], f32)
        cc_out = dram.tile([1, P], f32)
        nc.sync.dma_start(out=cc_in[:].rearrange("a b -> b a"), in_=mloc[:])
        nc.gpsimd.collective_compute(
            "AllReduce",
            Alu.max,
            replica_groups=[list(range(NCORES))],
            ins=[cc_in.opt()],
            outs=[cc_out.opt()],
        )
        grow = const.tile([1, P], f32)
        nc.sync.dma_start(out=grow[:], in_=cc_out[:])

        # sc columns: 0=gmax, 1=s, 2=inv0, 3=s*inv0, 4=2-s*inv0, 5=inv, 6=s
        sc = const.tile([1, 8], f32)
        nc.vector.reduce_max(sc[0:1, 0:1], grow[:], axis=X)
        nc.scalar.mul(sc[0:1, 1:2], sc[0:1, 0:1], 1.0 / QMAX_U)
        nc.vector.reciprocal(sc[0:1, 2:3], sc[0:1, 1:2])
        nc.vector.tensor_mul(sc[0:1, 3:4], sc[0:1, 1:2], sc[0:1, 2:3])
        nc.vector.tensor_scalar(
            out=sc[0:1, 4:5], in0=sc[0:1, 3:4],
            scalar1=-1.0, scalar2=2.0, op0=Alu.mult, op1=Alu.add,
        )
        nc.vector.tensor_mul(sc[0:1, 5:6], sc[0:1, 2:3], sc[0:1, 4:5])
        nc.vector.tensor_copy(sc[0:1, 6:7], sc[0:1, 1:2])

        # broadcast [inv, s] to all 128 partitions via a K=1 matmul with ones
        ones = const.tile([1, P], f32)
        nc.vector.memset(ones[:], 1.0)
        psc = psb.tile([P, 2], f32)
        nc.tensor.matmul(psc[:], ones[:], sc[0:1, 5:7], start=True, stop=True)
        scal = const.tile([P, 2], f32)
        nc.vector.tensor_copy(scal[:], psc[:])

        # ---- Phase B: out = round(y * inv) * s via magic-constant RNE
        for c in range(NCH):
            bb, j = divmod(c, NJ)
            t = tpool.tile([P, CH2], f32)
            nc.scalar.activation(
                t[:], ybig[:, c * CH2:(c + 1) * CH2], Copy,
                bias=MAGIC, scale=scal[:, 0:1],
            )
            nc.vector.tensor_scalar(
                out=t[:], in0=t[:],
                scalar1=-MAGIC, scalar2=scal[:, 1:2],
                op0=Alu.add, op1=Alu.mult,
            )
            # one 1 MiB DMA: [p, (m n)] -> [cout=(m p), n]
            nc.sync.dma_start(
                out=out[bb, :, j * NT:(j + 1) * NT].rearrange(
                    "(m p) n -> p m n", p=P
                ),
                in_=t[:, :].rearrange("p (m n) -> p m n", m=MT),
            )
    nc.compile()  # bacc lowering: register allocation, DCE, nop-fusion
    return nc


def _quant_po2(v, qmax):
    # mirrors reference.fake_quant_signed_po2 in float32
    v = np.asarray(v, np.float32)
    qmax = np.float32(qmax)
    maxabs = np.max(np.abs(v)).astype(np.float32)
    ratio = np.float32(maxabs / qmax)
    s = np.exp2(np.ceil(np.log2(ratio))).astype(np.float32)
    return (np.round(np.clip(v / s, -qmax, qmax)).astype(np.float32) * s).astype(
        np.float32
    )


def kernel(x, W, b):
    global LAST_RESULT
    x = np.ascontiguousarray(np.asarray(x, np.float32))
    W = np.asarray(W, np.float32)
    b = np.asarray(b, np.float32)
    assert x.shape == (B, CIN, N) and W.shape == (COUT, CIN) and b.shape == (COUT,)

    Wq = _quant_po2(W, QMAX_S)
    bq = _quant_po2(b, QMAX_S)
    wT_h = np.ascontiguousarray(Wq.T)                      # [CIN, COUT]
    bq_h = np.ascontiguousarray(bq.reshape(MT, P).T)       # [P, MT]

    if "nc" not in _cache:
        _cache["nc"] = _build()
    nc = _cache["nc"]

    in_maps = [
        {"xs": x[c * BSH:(c + 1) * BSH], "wT": wT_h, "bqv": bq_h}
        for c in range(NCORES)
    ]
    res = run_bass_kernel_spmd(nc, in_maps, core_ids=list(range(NCORES)))
    LAST_RESULT = res
    return np.concatenate(
        [res.results[c]["out"] for c in range(NCORES)], axis=0
    ).astype(np.float32)


if __name__ == "__main__":
    rng = np.random.default_rng(0)
    x = rng.standard_normal((B, CIN, N), np.float32)
    W = (rng.standard_normal((COUT, CIN)) * 0.05).astype(np.float32)
    b = (rng.standard_normal((COUT,)) * 0.1).astype(np.float32)
    y = kernel(x=x, W=W, b=b)
    print("out", y.shape, y.dtype, float(y.min()), float(y.max()))



# revision 8
# speedup vs baseline: 1.4695x; 1.4695x over previous
"""Fused ConvBNReLU1D (kernel_size=1) + per-tensor po2 weight/bias fake-quant
+ QuantReLU(8-bit unsigned) output fake-quant, on 8 Trainium2 NeuronCores.

Strategy (v2)
-------------
- Host: quantize W/b (per-tensor po2 scales - the "precomputed scale" option
  from the sharding hint). Wq's int8 levels are exact in fp16, so W ships as
  fp16; x ships as fp16 (|rel err| <= 2^-11, far inside the 2e-2 gate). The
  device returns fp16 outputs (values are 8-bit quantized anyway); host only
  gathers and upcasts.
- Device (SPMD, data-parallel over batch B=32 -> 4 batches/core):
  Phase A: y = relu(Wq @ x + bq) with fp16 matmuls (1 cycle/row), PSUM fp32,
           Relu+bias on the Act engine writing y as fp16 to SBUF (halves both
           SBUF footprint and later DVE cost); per-chunk running maxes on DVE.
  - AllGather of the per-partition max vector (cheaper than AllReduce in the
    collective cost model: no 1.875x multiplier), then a local 1024-wide max.
  Phase B: out = round(y/s)*s via the fp16 magic constant 1536 (=1.5*2^10):
           q+1536 is exactly representable for q in [0,255], so a single
           tensor_scalar (y*inv + 1536 -> fp16 RNE) performs round-to-nearest
           -even, and a second tensor_scalar maps t -> t*s - 1536*s = q*s.
           All phase-B operands are 2-byte SBUF tiles (DVE fast mode), and
           each chunk's 0.5 MiB fp16 result DMAs out as soon as it is ready.
"""

import os
import sys
from contextlib import ExitStack

import numpy as np

for _p in ("/opt/trn_rl_repo", os.path.expanduser("~/.axon_site/_ro/trn_rl_repo")):
    if os.path.isdir(_p) and _p not in sys.path:
        sys.path.insert(0, _p)

import concourse.bacc as bacc
import concourse.mybir as mybir
import concourse.tile as tile
from concourse.bass_utils import run_bass_kernel_spmd

P = 128
B, CIN, COUT, N = 32, 512, 512, 2048
NCORES = 8
BSH = B // NCORES          # batches per core
NT = 512                   # matmul free dim (= one PSUM bank of fp32)
KT = CIN // P              # 4 contraction tiles
MT = COUT // P             # 4 output-row tiles
NJ = N // NT               # 4 n-windows per batch
NCH = BSH * NJ             # 16 (batch, n-window) chunks per core
CH2 = MT * NT              # columns of y per chunk (2048)
MAGIC = 1536.0             # 1.5 * 2^10: fp16 RNE rounding for t in [0, 512)
QMAX_S = 127.0
QMAX_U = 255.0

_cache = {}
LAST_RESULT = None         # BassKernelResults of the most recent run (test.py)


def _build():
    f16 = mybir.dt.float16
    f32 = mybir.dt.float32
    Relu = mybir.ActivationFunctionType.Relu
    X = mybir.AxisListType.X
    Alu = mybir.AluOpType

    nc = bacc.Bacc(
        "TRN2",
        target_bir_lowering=False,
        debug=False,
        enable_asserts=False,
        num_devices=NCORES,
    )
    xs = nc.dram_tensor("xs", [BSH, CIN, N], f16, kind="ExternalInput")
    wimg = nc.dram_tensor("wimg", [P, MT * KT * P], f16, kind="ExternalInput")
    bqv = nc.dram_tensor("bqv", [P, MT], f32, kind="ExternalInput")
    out = nc.dram_tensor("out", [BSH, COUT, N], f16, kind="ExternalOutput")

    with tile.TileContext(nc) as tc, ExitStack() as ctx:
        const = ctx.enter_context(tc.tile_pool(name="const", bufs=1))
        xpool = ctx.enter_context(tc.tile_pool(name="xp", bufs=3))
        ypool = ctx.enter_context(tc.tile_pool(name="yp", bufs=1))
        pspool = ctx.enter_context(tc.tile_pool(name="ps", bufs=6, space="PSUM"))
        psb = ctx.enter_context(tc.tile_pool(name="psb", bufs=1, space="PSUM"))
        pswp = ctx.enter_context(tc.tile_pool(name="psw", bufs=1, space="PSUM"))
        tpool = ctx.enter_context(tc.tile_pool(name="tp", bufs=3))
        opool = ctx.enter_context(tc.tile_pool(name="op", bufs=3))
        dram = ctx.enter_context(tc.tile_pool(name="dram", bufs=1, space="DRAM"))

        wq = const.tile([P, MT * KT * P], f16)
        bias = const.tile([P, MT], f32)

        def load_x_chunk(c, split=False):
            bb, j = divmod(c, NJ)
            xt = xpool.tile([P, KT * NT], f16)
            if split:
                # chunk 0: per-k DMAs so matmul k waits only on its own slice
                for k in range(KT):
                    nc.sync.dma_start(
                        out=xt[:, k * NT:(k + 1) * NT],
                        in_=xs[bb, k * P:(k + 1) * P, j * NT:(j + 1) * NT],
                    )
            else:
                nc.sync.dma_start(
                    out=xt[:, :].rearrange("p (k n) -> p k n", k=KT),
                    in_=xs[bb, :, j * NT:(j + 1) * NT].rearrange(
                        "(k p) n -> p k n", p=P
                    ),
                )
            return xt

        # PE warm-up: a stream of dummy matmuls on a zeroed tile keeps the
        # p-state monitor busy from ~0.5us so the first real matmul (arriving
        # ~3us in, once weights+x land) already runs at the full 2.4 GHz
        warm = const.tile([1, 256], f16)
        nc.vector.memset(warm[:], 0.0)
        psw = pswp.tile([P, NT], f32)
        for _ in range(9):
            nc.tensor.matmul(
                psw[0:1, 0:256], warm[0:1, 0:1], warm[:], start=True, stop=True
            )

        # weights for m=0 first (matmul loop is m-outer), then x chunk 0,
        # then the remaining weight blocks; all fp16, host-prepared SBUF image
        nc.sync.dma_start(out=wq[:, 0:KT * P], in_=wimg[:, 0:KT * P])
        xtiles = {0: load_x_chunk(0, split=True)}
        for m in range(1, MT):
            nc.sync.dma_start(
                out=wq[:, m * KT * P:(m + 1) * KT * P],
                in_=wimg[:, m * KT * P:(m + 1) * KT * P],
            )
        nc.sync.dma_start(out=bias[:], in_=bqv[:, :])
        xtiles[1] = load_x_chunk(1)

        ybig = ypool.tile([P, NCH * CH2], f16)
        maxb = const.tile([P, NCH * MT], f32)

        # ---- Phase A: y = relu(Wq @ x + bq), track per-(chunk,m) maxes
        for c in range(NCH):
            xt = xtiles.pop(c) if c in xtiles else load_x_chunk(c)
            if c + 2 < NCH and c + 2 not in xtiles:
                xtiles[c + 2] = load_x_chunk(c + 2)
            for m in range(MT):
                ps = pspool.tile([P, NT], f32)
                for k in range(KT):
                    nc.tensor.matmul(
                        ps[:],
                        wq[:, (m * KT + k) * P:(m * KT + k + 1) * P],
                        xt[:, k * NT:(k + 1) * NT],
                        start=(k == 0),
                        stop=(k == KT - 1),
                    )
                col = c * CH2 + m * NT
                nc.scalar.activation(
                    ybig[:, col:col + NT], ps[:], Relu, bias=bias[:, m:m + 1]
                )
                nc.vector.reduce_max(
                    maxb[:, c * MT + m:c * MT + m + 1],
                    ybig[:, col:col + NT], axis=X,
                )

        # ---- Global max across cores (scale is global)
        mloc = const.tile([P, 1], f32)
        nc.vector.reduce_max(mloc[:], maxb[:], axis=X)
        cc_in = dram.tile([1, P], f32)
        cc_out = dram.tile([1, NCORES * P], f32)
        nc.sync.dma_start(out=cc_in[:].rearrange("a b -> b a"), in_=mloc[:])
        nc.gpsimd.collective_compute(
            "AllGather",
            Alu.bypass,
            replica_groups=[list(range(NCORES))],
            ins=[cc_in.opt()],
            outs=[cc_out.opt()],
        )
        grow = const.tile([1, NCORES * P], f32)
        nc.sync.dma_start(out=grow[:], in_=cc_out[:])

        # sc columns: 0=gmax, 1=s, 2=inv, 3=-1536*s; s and -1536*s both derive
        # directly from gmax so they don't serialize behind each other
        sc = const.tile([1, 4], f32)
        nc.vector.reduce_max(sc[0:1, 0:1], grow[:], axis=X)
        nc.vector.tensor_scalar(
            out=sc[0:1, 1:2], in0=sc[0:1, 0:1],
            scalar1=1.0 / QMAX_U, scalar2=0.0, op0=Alu.mult, op1=Alu.add,
        )
        nc.vector.tensor_scalar(
            out=sc[0:1, 3:4], in0=sc[0:1, 0:1],
            scalar1=-MAGIC / QMAX_U, scalar2=0.0, op0=Alu.mult, op1=Alu.add,
        )
        nc.vector.reciprocal_approx_fast(out=sc[0:1, 2:3], in_=sc[0:1, 1:2])

        # broadcast [s, inv, -1536s] to all 128 partitions via a K=1 matmul
        ones = const.tile([1, P], f32)
        nc.vector.memset(ones[:], 1.0)
        psc = psb.tile([P, 3], f32)
        nc.tensor.matmul(psc[:], ones[:], sc[0:1, 1:4], start=True, stop=True)
        scal = const.tile([P, 3], f32)
        nc.vector.tensor_copy(scal[:], psc[:])

        # ---- Phase B: t = RNE(y*inv) + 1536 (fp16 magic), out = t*s - 1536*s
        # first chunk at (c,m) granularity so the first out-DMA issues early
        def quant(src_lo, width, dst):
            t = tpool.tile([P, width], f16)
            nc.vector.tensor_scalar(
                out=t[:], in0=ybig[:, src_lo:src_lo + width],
                scalar1=scal[:, 1:2], scalar2=MAGIC,
                op0=Alu.mult, op1=Alu.add,
            )
            nc.vector.tensor_scalar(
                out=dst, in0=t[:],
                scalar1=scal[:, 0:1], scalar2=scal[:, 2:3],
                op0=Alu.mult, op1=Alu.add,
            )

        for c in range(NCH):
            bb, j = divmod(c, NJ)
            o = opool.tile([P, CH2], f16)
            if c == 0:
                for m in range(MT):
                    quant(c * CH2 + m * NT, NT, o[:, m * NT:(m + 1) * NT])
                    nc.sync.dma_start(
                        out=out[bb, m * P:(m + 1) * P, j * NT:(j + 1) * NT],
                        in_=o[:, m * NT:(m + 1) * NT],
                    )
                continue
            quant(c * CH2, CH2, o[:])
            # one 0.5 MiB DMA: [p, (m n)] -> [cout=(m p), n]
            nc.sync.dma_start(
                out=out[bb, :, j * NT:(j + 1) * NT].rearrange(
                    "(m p) n -> p m n", p=P
                ),
                in_=o[:, :].rearrange("p (m n) -> p m n", m=MT),
            )
    nc.compile()  # bacc lowering: register allocation, DCE, nop-fusion
    return nc


def _quant_po2(v, qmax):
    # mirrors reference.fake_quant_signed_po2 in float32
    v = np.asarray(v, np.float32)
    qmax = np.float32(qmax)
    maxabs = np.max(np.abs(v)).astype(np.float32)
    ratio = np.float32(maxabs / qmax)
    s = np.exp2(np.ceil(np.log2(ratio))).astype(np.float32)
    return (np.round(np.clip(v / s, -qmax, qmax)).astype(np.float32) * s).astype(
        np.float32
    )


def kernel(x, W, b):
    global LAST_RESULT
    x = np.asarray(x, np.float32)
    W = np.asarray(W, np.float32)
    b = np.asarray(b, np.float32)
    assert x.shape == (B, CIN, N) and W.shape == (COUT, CIN) and b.shape == (COUT,)

    x16 = np.ascontiguousarray(x.astype(np.float16))
    Wq = _quant_po2(W, QMAX_S)
    bq = _quant_po2(b, QMAX_S)
    # SBUF image: wq[p, (m*KT + k)*P + q] = Wq.T[k*P + p, m*P + q]
    wimg_h = np.ascontiguousarray(
        Wq.T.reshape(KT, P, MT, P).transpose(1, 2, 0, 3).reshape(P, MT * KT * P)
        .astype(np.float16)
    )
    bq_h = np.ascontiguousarray(bq.reshape(MT, P).T)       # [P, MT]

    if "nc" not in _cache:
        _cache["nc"] = _build()
    nc = _cache["nc"]

    in_maps = [
        {"xs": x16[c * BSH:(c + 1) * BSH], "wimg": wimg_h, "bqv": bq_h}
        for c in range(NCORES)
    ]
    res = run_bass_kernel_spmd(nc, in_maps, core_ids=list(range(NCORES)))
    LAST_RESULT = res
    return np.concatenate(
        [res.results[c]["out"] for c in range(NCORES)], axis=0
    ).astype(np.float32)


if __name__ == "__main__":
    rng = np.random.default_rng(0)
    x = rng.standard_normal((B, CIN, N), np.float32)
    W = (rng.standard_normal((COUT, CIN)) * 0.05).astype(np.float32)
    b = (rng.standard_normal((COUT,)) * 0.1).astype(np.float32)
    y = kernel(x=x, W=W, b=b)
    print("out", y.shape, y.dtype, float(y.min()), float(y.max()))
